# revision 13
# baseline (speedup 1.0000x reference)
"""DirectedEncoder GNN kernel for 8 Trainium2 NeuronCores.

out = ALPHA*(segment_sum(x[edge_src] by edge_dst) @ W_sd.T + b_sd)
    + (1-ALPHA)*(segment_sum(x[edge_dst] by edge_src) @ W_ds.T + b_ds)

Sharding: edges are grouped by destination node (direction 2 by source), and
destination nodes are range-sharded across the 8 cores, so each core owns a
disjoint slice of output rows and no cross-core reduction is needed.

Per core the kernel:
  - gathers x rows per edge with dma_gather (int16 indices relative to one of
    four 25088-row source ranges, 4 SWDGE queues in parallel). Measured cost
    per gathered row ~= 0.82 ns fixed + bytes/385 GB/s (aggregate), plus
    ~0.76 us fixed per dma_gather call, and the gather dominates the whole
    kernel. So: rows are plain bf16 (256 B payloads beat 512 B duplicated
    pairs), and both directions + all windows of a 1024-dst macro share one
    gather call per source range (52 calls instead of 200),
  - converts each 128-edge chunk into a one-hot [128 edges x 128 dst] bf16
    matrix on the vector engine (tensor_tensor is_equal against an iota
    constant),
  - matmul-accumulates transposed aggregates aggT[feat, dst] into a
    [128, 1024] fp32 PSUM region with bf16 1-cycle/row matmuls (chunks are
    window-pure: host pads every (dir, macro, window-of-128, range) edge
    group to a multiple of 128 using the max count across cores, so one
    compiled SPMD program serves all 8 cores),
  - projects with the pre-transposed, ALPHA-folded bf16 weights and adds the
    combined fp32 bias via the scalar engine,
  - stores the output transposed [128 feat, nodes] fp32; the host
    reassembles.

USE_FP32=True restores the all-fp32 variant; USE_DUP=True restores 512 B
duplicated rows; micro.py uses the module-level knobs for A/B timing.
"""

from dataclasses import dataclass, field

import ml_dtypes
import numpy as np

import concourse.mybir as mybir
import concourse.tile as tile
from concourse import bacc
from concourse.bass_utils import run_bass_kernel_spmd

BF16 = ml_dtypes.bfloat16

P = 128
NCORE = 8
MACRO = 512
ALPHA = 0.5

NQUEUES = 4
SCRATCH = 81920
XG_BUFS = 2
OH_BUFS = 8

# micro.py bottleneck decomposition knobs
SKIP_GATHER = False
SKIP_COMPUTE = False
SINGLE_PACKET = False
USE_FP32 = False
USE_DUP = False  # duplicate bf16 rows to 512 B payloads


def roundup(a, b):
    return (a + b - 1) // b * b


@dataclass
class Meta:
    n_nodes: int
    span: int
    nmacro: int
    nw: int
    rspan: int
    nrange: int
    xrows: int
    totc16: int = 0
    totch: int = 0
    cmax: int = 0
    reps: int = 1
    sched: list = field(default_factory=list)


def prep(x, edge_src, edge_dst):
    n = x.shape[0]
    span = roundup((n + NCORE - 1) // NCORE, P)
    nmacro = (span + MACRO - 1) // MACRO
    nw = MACRO // P
    xrows = roundup(n, 2048)
    nrange = 4
    rspan = xrows // nrange
    assert rspan <= 32768 and nrange * rspan == xrows
    meta = Meta(n_nodes=n, span=span, nmacro=nmacro, nw=nw, rspan=rspan,
                nrange=nrange, xrows=xrows)

    if USE_FP32:
        x_pad = np.zeros((xrows, P), dtype=np.float32)
        x_pad[:n] = x
    elif USE_DUP:
        x_pad = np.zeros((xrows, 2 * P), dtype=BF16)
        x_pad[:n, :P] = x.astype(BF16)
        x_pad[:n, P:] = x_pad[:n, :P]
    else:
        x_pad = np.zeros((xrows, P), dtype=BF16)
        x_pad[:n] = x.astype(BF16)

    key = [np.asarray(edge_dst, np.int64), np.asarray(edge_src, np.int64)]
    gat = [np.asarray(edge_src, np.int64), np.asarray(edge_dst, np.int64)]

    counts = np.zeros((2, NCORE, nmacro, nw, nrange), dtype=np.int64)
    edge_groups = []
    for d in range(2):
        k, g = key[d], gat[d]
        core = k // span
        local = k - core * span
        m = local // MACRO
        w = (local % MACRO) // P
        r = g // rspan
        np.add.at(counts[d], (core, m, w, r), 1)
        comp = ((core * nmacro + m) * nw + w) * nrange + r
        order = np.argsort(comp, kind="stable")
        cs = comp[order]
        uniq, starts = np.unique(cs, return_index=True)
        ends = np.append(starts[1:], len(cs))
        edge_groups.append({int(u): order[s:e]
                            for u, s, e in zip(uniq, starts, ends)})

    gmax = counts.max(axis=1)
    G = np.where(gmax > 0, roundup(gmax, P), 0)  # [2, nmacro, nw, nrange]

    meta.totch = int(G.sum() // P)
    meta.totc16 = int(G.sum() // 16)
    meta.cmax = max(int(G[:, m].sum() // P) for m in range(nmacro))

    idx16 = np.zeros((NCORE, P, meta.totc16), dtype=np.int16)
    NPDT = np.float32 if USE_FP32 else BF16
    dstloc = np.full((NCORE, P, meta.totch), -1.0, dtype=NPDT)

    col16 = 0
    slot = 0
    sched = []
    for m in range(nmacro):
        gathers = []
        per_d = {0: [], 1: []}  # (crel0, w, nchg, dmat[NCORE, P, nchg])
        crel = 0
        for r in range(nrange):
            Gmr = int(G[:, m, :, r].sum())
            if Gmr == 0:
                continue
            crel_call = crel
            gathers.append((r, col16, crel_call, Gmr // P))
            for d in range(2):
                k, g = key[d], gat[d]
                for w in range(nw):
                    Gs = int(G[d, m, w, r])
                    if Gs == 0:
                        continue
                    dmat = np.full((NCORE, P, Gs // P), -1.0, dtype=NPDT)
                    for c in range(NCORE):
                        u = ((c * nmacro + m) * nw + w) * nrange + r
                        eids = edge_groups[d].get(u, np.empty(0, np.int64))
                        # ascending gather addresses within the group improve
                        # HBM row/bank locality
                        eids = eids[np.argsort(g[eids], kind="stable")]
                        cnt = len(eids)
                        flat_i = np.zeros(Gs, dtype=np.int16)
                        flat_i[:cnt] = (g[eids] - r * rspan).astype(np.int16)
                        flat_d = np.full(Gs, -1.0, dtype=NPDT)
                        flat_d[:cnt] = (k[eids] % P).astype(NPDT)
                        wrapped = flat_i.reshape(Gs // 16, 16).T
                        c16 = col16 + (crel - crel_call) * (P // 16)
                        idx16[c, :, c16:c16 + Gs // 16] = np.tile(
                            wrapped, (8, 1))
                        dmat[c] = flat_d.reshape(Gs // P, P).T
                    per_d[d].append((crel, w, Gs // P, dmat))
                    crel += Gs // P
            col16 += Gmr // 16
        chunks = {0: [], 1: []}
        for d in range(2):
            for crel0, w, nchg, dmat in per_d[d]:
                dstloc[:, :, slot:slot + nchg] = dmat
                for j in range(nchg):
                    chunks[d].append((crel0 + j, w))
                slot += nchg
        sched.append({"gathers": gathers, "chunks": chunks})
    assert col16 == meta.totc16 and slot == meta.totch
    meta.sched = sched
    return meta, x_pad, idx16, dstloc


def build(meta: Meta):
    nc = bacc.Bacc("TRN2", target_bir_lowering=False,
                   num_swdge_queues=NQUEUES,
                   dynamic_dma_scratch_size=SCRATCH)
    OUTW = meta.nmacro * MACRO
    MDT = mybir.dt.float32 if USE_FP32 else mybir.dt.bfloat16
    XW = P if (USE_FP32 or not USE_DUP) else 2 * P

    x_t = nc.dram_tensor("x_pad", [meta.xrows, XW], MDT,
                         kind="ExternalInput")
    idx_d = nc.dram_tensor("idx16", [P, meta.totc16], mybir.dt.int16,
                           kind="ExternalInput")
    dl_d = nc.dram_tensor("dstloc", [P, meta.totch], MDT,
                          kind="ExternalInput")
    iota_d = nc.dram_tensor("iota_c", [P, 4 * P], MDT,
                            kind="ExternalInput")
    w1_d = nc.dram_tensor("w1t", [P, P], MDT, kind="ExternalInput")
    w2_d = nc.dram_tensor("w2t", [P, P], MDT, kind="ExternalInput")
    b_d = nc.dram_tensor("biasc", [P, 1], mybir.dt.float32, kind="ExternalInput")
    out_d = nc.dram_tensor("out_t", [P, OUTW], mybir.dt.float32,
                           kind="ExternalOutput")

    with tile.TileContext(nc) as tc:
        with (
            tc.tile_pool(name="consts", bufs=1) as consts,
            tc.tile_pool(name="xg", bufs=XG_BUFS) as xgp,
            tc.tile_pool(name="oh", bufs=OH_BUFS) as ohp,
            tc.tile_pool(name="sb", bufs=2) as sbp,
            tc.tile_pool(name="ps", bufs=2 if MACRO <= 512 else 1,
                         space="PSUM") as psp,
        ):
            iota_t = consts.tile([P, 4, P], MDT)
            nc.sync.dma_start(iota_t[:].rearrange("p c f -> p (c f)"),
                              iota_d[:])
            w1_t = consts.tile([P, P], MDT)
            nc.sync.dma_start(w1_t[:], w1_d[:])
            w2_t = consts.tile([P, P], MDT)
            nc.sync.dma_start(w2_t[:], w2_d[:])
            b_t = consts.tile([P, 1], mybir.dt.float32)
            nc.sync.dma_start(b_t[:], b_d[:])
            zero_t = consts.tile([P, MACRO], MDT)
            nc.vector.memset(zero_t[:], 0.0)
            dl_t = consts.tile([P, meta.totch], MDT)
            nc.sync.dma_start(dl_t[:], dl_d[:])

            idxcols_m = []
            for m in range(meta.nmacro):
                c0 = min(g[1] for g in meta.sched[m]["gathers"])
                c1 = max(g[1] + g[3] * 8 for g in meta.sched[m]["gathers"])
                idxcols_m.append((c0, c1))
            idxw = max(c1 - c0 for c0, c1 in idxcols_m)

            for _rep in range(meta.reps):
                slot = 0
                for m in range(meta.nmacro):
                    mc0, mc1 = idxcols_m[m]
                    idx_t = sbp.tile([P, idxw], mybir.dt.int16, tag="idx")
                    nc.sync.dma_start(idx_t[:, :mc1 - mc0],
                                      idx_d[:, mc0:mc1])
                    sch = meta.sched[m]
                    xg = xgp.tile([P, meta.cmax, XW], MDT, tag="xg")
                    if SKIP_GATHER:
                        nc.vector.memset(xg[:, 0, 0:1], 0.0)
                    for gi, (r, col16, crel0, nch) in enumerate(
                            sch["gathers"]):
                        if SKIP_GATHER:
                            continue
                        Gmr = nch * P
                        nc.gpsimd.dma_gather(
                            xg[:, crel0:crel0 + nch, :],
                            x_t[r * meta.rspan:(r + 1) * meta.rspan, :],
                            idx_t[:, col16 - mc0:col16 - mc0 + Gmr // 16],
                            Gmr, Gmr, XW, single_packet=SINGLE_PACKET,
                            queue_num=gi % NQUEUES)
                    if SKIP_COMPUTE:
                        continue
                    nhalf = meta.nw // 4
                    agg_sb = []
                    for d in range(2):
                        chunks = sch["chunks"][d]
                        C = len(chunks)
                        # last chunk index per 512-wide PSUM half
                        last_ci = {}
                        for ci, (crel, w) in enumerate(chunks):
                            last_ci[w // 4] = ci
                        aggs = []
                        for h in range(nhalf):
                            agg_ps = psp.tile([P, 512], mybir.dt.float32,
                                              space="PSUM", tag=f"agg{d}h{h}")
                            nc.tensor.matmul(agg_ps[:], zero_t[:, :P],
                                             zero_t[:, :512],
                                             start=True,
                                             stop=h not in last_ci)
                            aggs.append(agg_ps)
                        for b0 in range(0, C, 4):
                            bn = min(4, C - b0)
                            oh4 = ohp.tile([P, 4, P], MDT,
                                           tag="oh")
                            nc.vector.tensor_tensor(
                                out=oh4[:, :bn, :], in0=iota_t[:, :bn, :],
                                in1=dl_t[:, slot:slot + bn].to_broadcast(
                                    [P, bn, P]),
                                op=mybir.AluOpType.is_equal)
                            for kk in range(bn):
                                ci = b0 + kk
                                crel, w = chunks[ci]
                                nc.tensor.matmul(
                                    aggs[w // 4][:, (w % 4) * P:
                                                 (w % 4 + 1) * P],
                                    xg[:, crel, :P], oh4[:, kk, :],
                                    start=False,
                                    stop=(ci == last_ci[w // 4]))
                            slot += bn
                        a_sb = sbp.tile([P, MACRO], MDT,
                                        tag=f"agg_sb{d}")
                        for h in range(nhalf):
                            nc.scalar.activation(
                                out=a_sb[:, h * 512:(h + 1) * 512],
                                in_=aggs[h][:],
                                func=mybir.ActivationFunctionType.Copy)
                        agg_sb.append(a_sb)

                    o_sb = sbp.tile([P, MACRO], mybir.dt.float32, tag="o_sb")
                    for h in range(nhalf):
                        out_ps = psp.tile([P, 512], mybir.dt.float32,
                                          space="PSUM", tag=f"out_ps{h}")
                        nc.tensor.matmul(
                            out_ps[:], w1_t[:],
                            agg_sb[0][:, h * 512:(h + 1) * 512],
                            start=True, stop=False)
                        nc.tensor.matmul(
                            out_ps[:], w2_t[:],
                            agg_sb[1][:, h * 512:(h + 1) * 512],
                            start=False, stop=True)
                        nc.scalar.activation(
                            out=o_sb[:, h * 512:(h + 1) * 512], in_=out_ps[:],
                            func=mybir.ActivationFunctionType.Identity,
                            bias=b_t[:, :1])
                    nc.sync.dma_start(out_d[:, m * MACRO:(m + 1) * MACRO],
                                      o_sb[:])

    nc.compile()
    return nc


def make_inputs(meta, x_pad, idx16, dstloc, W_sd, b_sd, W_ds, b_ds):
    NPDT = np.float32 if USE_FP32 else BF16
    w1t = np.ascontiguousarray((ALPHA * np.asarray(W_sd)).T.astype(NPDT))
    w2t = np.ascontiguousarray(
        ((1.0 - ALPHA) * np.asarray(W_ds)).T.astype(NPDT))
    biasc = (ALPHA * np.asarray(b_sd)
             + (1.0 - ALPHA) * np.asarray(b_ds)).astype(np.float32)[:, None]
    iota_c = np.tile(np.arange(P, dtype=NPDT), (P, 4))
    return [{
        "x_pad": x_pad, "idx16": np.ascontiguousarray(idx16[c]),
        "dstloc": np.ascontiguousarray(dstloc[c]),
        "iota_c": iota_c, "w1t": w1t, "w2t": w2t,
        "biasc": np.ascontiguousarray(biasc),
    } for c in range(NCORE)]


def assemble(meta, results):
    parts = [results[c]["out_t"][:, :meta.span].T for c in range(NCORE)]
    return np.ascontiguousarray(
        np.concatenate(parts, axis=0)[:meta.n_nodes])


def kernel(x, edge_src, edge_dst, W_sd, b_sd, W_ds, b_ds):
    meta, x_pad, idx16, dstloc = prep(
        np.asarray(x, dtype=np.float32), edge_src, edge_dst)
    nc = build(meta)
    in_maps = make_inputs(meta, x_pad, idx16, dstloc, W_sd, b_sd, W_ds, b_ds)
    res = run_bass_kernel_spmd(nc, in_maps, core_ids=list(range(NCORE)))
    return assemble(meta, res.results)


# revision 17
# speedup vs baseline: 1.0528x; 1.0528x over previous
"""DirectedEncoder GNN kernel for 8 Trainium2 NeuronCores.

out = ALPHA*(segment_sum(x[edge_src] by edge_dst) @ W_sd.T + b_sd)
    + (1-ALPHA)*(segment_sum(x[edge_dst] by edge_src) @ W_ds.T + b_ds)

Sharding: edges are grouped by destination node (direction 2 by source), and
destination nodes are range-sharded across the 8 cores, so each core owns a
disjoint slice of output rows and no cross-core reduction is needed.

The whole kernel is bounded by SWDGE gather descriptor generation on the Q7
(Pool) engine: measured ~1.6 ns/descriptor aggregate plus ~0.76 us fixed per
dma_gather call, with the DMA transfers hidden behind generation when all 4
SWDGE queues are used (2 queues doubles runtime). Hence:
  - x rows are plain bf16 (256 B payloads; transfers stay under the gen
    rate, unlike 512 B duplicated rows which stall the queues),
  - macros are 1024 destinations wide so each (direction, macro, range)
    gather is one big call: 104 calls instead of 200,
  - per-direction gather buffers keep compute of direction 0 overlapped
    with direction 1's transfers.

Per core the kernel:
  - gathers x rows per edge with dma_gather (int16 indices relative to one
    of four 25088-row source ranges, 4 SWDGE queues round-robin),
  - converts each 128-edge chunk into a one-hot [128 edges x 128 dst] bf16
    matrix on the vector engine (tensor_tensor is_equal against an iota
    constant),
  - matmul-accumulates transposed aggregates aggT[feat, dst] into two
    [128, 512] fp32 PSUM banks per direction with bf16 1-cycle/row matmuls
    (chunks are window-pure: host pads every (dir, macro, window-of-128,
    range) edge group to a multiple of 128 using the max count across
    cores, so one compiled SPMD program serves all 8 cores),
  - projects with the pre-transposed, ALPHA-folded bf16 weights and adds
    the combined fp32 bias via the scalar engine,
  - stores the output transposed [128 feat, nodes] fp32; the host
    reassembles.
"""

from dataclasses import dataclass, field

import ml_dtypes
import numpy as np

import concourse.mybir as mybir
import concourse.tile as tile
from concourse import bacc
from concourse.bass_utils import run_bass_kernel_spmd

BF16 = ml_dtypes.bfloat16

P = 128
NCORE = 8
MACRO = 1024
ALPHA = 0.5

NQUEUES = 4
SCRATCH = 65536
XG_BUFS = 2
OH_BUFS = 8

# micro.py bottleneck decomposition knobs
SKIP_GATHER = False
SKIP_COMPUTE = False
SINGLE_PACKET = False
USE_FP32 = False
USE_DUP = False  # duplicate bf16 rows to 512 B payloads


def roundup(a, b):
    return (a + b - 1) // b * b


@dataclass
class Meta:
    n_nodes: int
    span: int
    nmacro: int
    nw: int
    rspan: int
    nrange: int
    xrows: int
    totc16: int = 0
    totch: int = 0
    cmax: int = 0
    reps: int = 1
    sched: list = field(default_factory=list)


def prep(x, edge_src, edge_dst):
    n = x.shape[0]
    span = roundup((n + NCORE - 1) // NCORE, P)
    nmacro = (span + MACRO - 1) // MACRO
    nw = MACRO // P
    xrows = roundup(n, 2048)
    nrange = 4
    rspan = xrows // nrange
    assert rspan <= 32768 and nrange * rspan == xrows
    meta = Meta(n_nodes=n, span=span, nmacro=nmacro, nw=nw, rspan=rspan,
                nrange=nrange, xrows=xrows)

    if USE_FP32:
        x_pad = np.zeros((xrows, P), dtype=np.float32)
        x_pad[:n] = x
    elif USE_DUP:
        x_pad = np.zeros((xrows, 2 * P), dtype=BF16)
        x_pad[:n, :P] = x.astype(BF16)
        x_pad[:n, P:] = x_pad[:n, :P]
    else:
        x_pad = np.zeros((xrows, P), dtype=BF16)
        x_pad[:n] = x.astype(BF16)

    key = [np.asarray(edge_dst, np.int64), np.asarray(edge_src, np.int64)]
    gat = [np.asarray(edge_src, np.int64), np.asarray(edge_dst, np.int64)]

    counts = np.zeros((2, NCORE, nmacro, nw, nrange), dtype=np.int64)
    edge_groups = []
    for d in range(2):
        k, g = key[d], gat[d]
        core = k // span
        local = k - core * span
        m = local // MACRO
        w = (local % MACRO) // P
        r = g // rspan
        np.add.at(counts[d], (core, m, w, r), 1)
        comp = ((core * nmacro + m) * nw + w) * nrange + r
        order = np.argsort(comp, kind="stable")
        cs = comp[order]
        uniq, starts = np.unique(cs, return_index=True)
        ends = np.append(starts[1:], len(cs))
        edge_groups.append({int(u): order[s:e]
                            for u, s, e in zip(uniq, starts, ends)})

    gmax = counts.max(axis=1)
    G = np.where(gmax > 0, roundup(gmax, P), 0)  # [2, nmacro, nw, nrange]

    meta.totch = int(G.sum() // P)
    meta.totc16 = int(G.sum() // 16)
    meta.cmax = max(int(G[d, m].sum() // P)
                    for d in range(2) for m in range(nmacro))

    idx16 = np.zeros((NCORE, P, meta.totc16), dtype=np.int16)
    NPDT = np.float32 if USE_FP32 else BF16
    dstloc = np.full((NCORE, P, meta.totch), -1.0, dtype=NPDT)

    col16 = 0
    slot = 0
    sched = []
    for m in range(nmacro):
        byd = []
        for d in range(2):
            k, g = key[d], gat[d]
            gathers = []
            chunks = []
            crel = 0
            for r in range(nrange):
                Gmr = int(G[d, m, :, r].sum())
                if Gmr == 0:
                    continue
                gathers.append((r, col16, crel, Gmr // P))
                for w in range(nw):
                    Gs = int(G[d, m, w, r])
                    if Gs == 0:
                        continue
                    for c in range(NCORE):
                        u = ((c * nmacro + m) * nw + w) * nrange + r
                        eids = edge_groups[d].get(u, np.empty(0, np.int64))
                        # ascending gather addresses within the group improve
                        # HBM row/bank locality
                        eids = eids[np.argsort(g[eids], kind="stable")]
                        cnt = len(eids)
                        flat_i = np.zeros(Gs, dtype=np.int16)
                        flat_i[:cnt] = (g[eids] - r * rspan).astype(np.int16)
                        flat_d = np.full(Gs, -1.0, dtype=NPDT)
                        flat_d[:cnt] = (k[eids] % P).astype(NPDT)
                        wrapped = flat_i.reshape(Gs // 16, 16).T
                        c16 = col16 + (crel * 128 - gathers[-1][2] * 128) // 16
                        idx16[c, :, c16:c16 + Gs // 16] = np.tile(
                            wrapped, (8, 1))
                        dstloc[c, :, slot:slot + Gs // P] = (
                            flat_d.reshape(Gs // P, P).T)
                    for _ in range(Gs // P):
                        chunks.append((crel, w))
                        crel += 1
                    slot += Gs // P
                col16 += Gmr // 16
            byd.append({"gathers": gathers, "chunks": chunks})
        sched.append(byd)
    assert col16 == meta.totc16 and slot == meta.totch
    meta.sched = sched
    return meta, x_pad, idx16, dstloc


def build(meta: Meta):
    nc = bacc.Bacc("TRN2", target_bir_lowering=False,
                   num_swdge_queues=NQUEUES,
                   dynamic_dma_scratch_size=SCRATCH)
    OUTW = meta.nmacro * MACRO
    MDT = mybir.dt.float32 if USE_FP32 else mybir.dt.bfloat16
    XW = P if (USE_FP32 or not USE_DUP) else 2 * P
    nhalf = MACRO // 512

    x_t = nc.dram_tensor("x_pad", [meta.xrows, XW], MDT,
                         kind="ExternalInput")
    idx_d = nc.dram_tensor("idx16", [P, meta.totc16], mybir.dt.int16,
                           kind="ExternalInput")
    dl_d = nc.dram_tensor("dstloc", [P, meta.totch], MDT,
                          kind="ExternalInput")
    iota_d = nc.dram_tensor("iota_c", [P, 4 * P], MDT,
                            kind="ExternalInput")
    w1_d = nc.dram_tensor("w1t", [P, P], MDT, kind="ExternalInput")
    w2_d = nc.dram_tensor("w2t", [P, P], MDT, kind="ExternalInput")
    b_d = nc.dram_tensor("biasc", [P, 1], mybir.dt.float32, kind="ExternalInput")
    out_d = nc.dram_tensor("out_t", [P, OUTW], mybir.dt.float32,
                           kind="ExternalOutput")

    with tile.TileContext(nc) as tc:
        with (
            tc.tile_pool(name="consts", bufs=1) as consts,
            tc.tile_pool(name="xg0", bufs=XG_BUFS) as xgp0,
            tc.tile_pool(name="xg1", bufs=XG_BUFS) as xgp1,
            tc.tile_pool(name="oh", bufs=OH_BUFS) as ohp,
            tc.tile_pool(name="sb", bufs=2) as sbp,
            tc.tile_pool(name="ps", bufs=2 if nhalf == 1 else 1,
                         space="PSUM") as psp,
        ):
            iota_t = consts.tile([P, 4, P], MDT)
            nc.sync.dma_start(iota_t[:].rearrange("p c f -> p (c f)"),
                              iota_d[:])
            w1_t = consts.tile([P, P], MDT)
            nc.sync.dma_start(w1_t[:], w1_d[:])
            w2_t = consts.tile([P, P], MDT)
            nc.sync.dma_start(w2_t[:], w2_d[:])
            b_t = consts.tile([P, 1], mybir.dt.float32)
            nc.sync.dma_start(b_t[:], b_d[:])
            zero_t = consts.tile([P, 512], MDT)
            nc.vector.memset(zero_t[:], 0.0)
            dl_t = consts.tile([P, meta.totch], MDT)
            nc.sync.dma_start(dl_t[:], dl_d[:])

            idxcols_m = []
            for m in range(meta.nmacro):
                c0 = min(g[1] for d in range(2)
                         for g in meta.sched[m][d]["gathers"])
                c1 = max(g[1] + g[3] * 8 for d in range(2)
                         for g in meta.sched[m][d]["gathers"])
                idxcols_m.append((c0, c1))
            idxw = max(c1 - c0 for c0, c1 in idxcols_m)

            for _rep in range(meta.reps):
                slot = 0
                for m in range(meta.nmacro):
                    mc0, mc1 = idxcols_m[m]
                    idx_t = sbp.tile([P, idxw], mybir.dt.int16, tag="idx")
                    nc.sync.dma_start(idx_t[:, :mc1 - mc0],
                                      idx_d[:, mc0:mc1])
                    xgs = []
                    for d in range(2):
                        sch = meta.sched[m][d]
                        xgp = xgp0 if d == 0 else xgp1
                        xg = xgp.tile([P, meta.cmax, XW], MDT,
                                      tag=f"xg{d}")
                        xgs.append(xg)
                        if SKIP_GATHER:
                            nc.vector.memset(xg[:, 0, 0:1], 0.0)
                        for gi, (r, col16, crel0, nch) in enumerate(
                                sch["gathers"]):
                            if SKIP_GATHER:
                                continue
                            Gmr = nch * P
                            nc.gpsimd.dma_gather(
                                xg[:, crel0:crel0 + nch, :],
                                x_t[r * meta.rspan:(r + 1) * meta.rspan, :],
                                idx_t[:, col16 - mc0:col16 - mc0 + Gmr // 16],
                                Gmr, Gmr, XW, single_packet=SINGLE_PACKET,
                                queue_num=(2 * m + d + gi) % NQUEUES)
                    if SKIP_COMPUTE:
                        continue
                    agg_sb = []
                    for d in range(2):
                        sch = meta.sched[m][d]
                        xg = xgs[d]
                        chunks = sch["chunks"]
                        C = len(chunks)
                        last_ci = {}
                        for ci, (crel, w) in enumerate(chunks):
                            last_ci[w // 4] = ci
                        aggs = []
                        for h in range(nhalf):
                            agg_ps = psp.tile([P, 512], mybir.dt.float32,
                                              space="PSUM", tag=f"agg{d}h{h}")
                            nc.tensor.matmul(agg_ps[:], zero_t[:, :P],
                                             zero_t[:],
                                             start=True,
                                             stop=h not in last_ci)
                            aggs.append(agg_ps)
                        for b0 in range(0, C, 4):
                            bn = min(4, C - b0)
                            oh4 = ohp.tile([P, 4, P], MDT,
                                           tag="oh")
                            nc.vector.tensor_tensor(
                                out=oh4[:, :bn, :], in0=iota_t[:, :bn, :],
                                in1=dl_t[:, slot:slot + bn].to_broadcast(
                                    [P, bn, P]),
                                op=mybir.AluOpType.is_equal)
                            for kk in range(bn):
                                ci = b0 + kk
                                crel, w = chunks[ci]
                                nc.tensor.matmul(
                                    aggs[w // 4][:, (w % 4) * P:
                                                 (w % 4 + 1) * P],
                                    xg[:, crel, :P], oh4[:, kk, :],
                                    start=False,
                                    stop=(ci == last_ci[w // 4]))
                            slot += bn
                        a_sb = sbp.tile([P, MACRO], MDT,
                                        tag=f"agg_sb{d}")
                        for h in range(nhalf):
                            nc.scalar.activation(
                                out=a_sb[:, h * 512:(h + 1) * 512],
                                in_=aggs[h][:],
                                func=mybir.ActivationFunctionType.Copy)
                        agg_sb.append(a_sb)

                    o_sb = sbp.tile([P, MACRO], mybir.dt.float32, tag="o_sb")
                    for h in range(nhalf):
                        out_ps = psp.tile([P, 512], mybir.dt.float32,
                                          space="PSUM", tag=f"out_ps{h}")
                        nc.tensor.matmul(
                            out_ps[:], w1_t[:],
                            agg_sb[0][:, h * 512:(h + 1) * 512],
                            start=True, stop=False)
                        nc.tensor.matmul(
                            out_ps[:], w2_t[:],
                            agg_sb[1][:, h * 512:(h + 1) * 512],
                            start=False, stop=True)
                        nc.scalar.activation(
                            out=o_sb[:, h * 512:(h + 1) * 512], in_=out_ps[:],
                            func=mybir.ActivationFunctionType.Identity,
                            bias=b_t[:, :1])
                    nc.sync.dma_start(out_d[:, m * MACRO:(m + 1) * MACRO],
                                      o_sb[:])

    nc.compile()
    return nc


def make_inputs(meta, x_pad, idx16, dstloc, W_sd, b_sd, W_ds, b_ds):
    NPDT = np.float32 if USE_FP32 else BF16
    w1t = np.ascontiguousarray((ALPHA * np.asarray(W_sd)).T.astype(NPDT))
    w2t = np.ascontiguousarray(
        ((1.0 - ALPHA) * np.asarray(W_ds)).T.astype(NPDT))
    biasc = (ALPHA * np.asarray(b_sd)
             + (1.0 - ALPHA) * np.asarray(b_ds)).astype(np.float32)[:, None]
    iota_c = np.tile(np.arange(P, dtype=NPDT), (P, 4))
    return [{
        "x_pad": x_pad, "idx16": np.ascontiguousarray(idx16[c]),
        "dstloc": np.ascontiguousarray(dstloc[c]),
        "iota_c": iota_c, "w1t": w1t, "w2t": w2t,
        "biasc": np.ascontiguousarray(biasc),
    } for c in range(NCORE)]


def assemble(meta, results):
    parts = [results[c]["out_t"][:, :meta.span].T for c in range(NCORE)]
    return np.ascontiguousarray(
        np.concatenate(parts, axis=0)[:meta.n_nodes])


def kernel(x, edge_src, edge_dst, W_sd, b_sd, W_ds, b_ds):
    meta, x_pad, idx16, dstloc = prep(
        np.asarray(x, dtype=np.float32), edge_src, edge_dst)
    nc = build(meta)
    in_maps = make_inputs(meta, x_pad, idx16, dstloc, W_sd, b_sd, W_ds, b_ds)
    res = run_bass_kernel_spmd(nc, in_maps, core_ids=list(range(NCORE)))
    return assemble(meta, res.results)


# revision 19
# speedup vs baseline: 1.1397x; 1.0826x over previous
"""DirectedEncoder GNN kernel for 8 Trainium2 NeuronCores.

out = ALPHA*(segment_sum(x[edge_src] by edge_dst) @ W_sd.T + b_sd)
    + (1-ALPHA)*(segment_sum(x[edge_dst] by edge_src) @ W_ds.T + b_ds)

Sharding: edges are grouped by destination node (direction 2 by source),
and destination nodes are assigned to the 8 cores by degree-sorted
round-robin dealing (node with global degree rank i goes to core i%8, slot
i//8), so each core owns a disjoint, degree-balanced set of output rows
and no cross-core reduction is needed. The host un-permutes the output.

The kernel is bounded by SWDGE gather descriptor generation on the Q7
(Pool) engine: ~1.6 ns/descriptor aggregate plus ~0.76 us per dma_gather
call, with DMA transfers hidden behind generation when all 4 SWDGE queues
are used (2 queues doubles runtime; 512 B duplicated rows stall queues).
Descriptor count therefore is the metric to minimize: rows are plain bf16
256 B payloads, and every (dir, macro, window-of-128, range) edge group is
padded to a multiple of only 16 indices (the idx16 wrap quantum), using
the max count across cores — degree balancing keeps that max tight. One
compiled SPMD program serves all 8 cores.

Per core the kernel:
  - gathers x rows per edge with dma_gather (int16 indices relative to one
    of four 25088-row source ranges, one call per (dir, macro, range), 4
    SWDGE queues round-robin),
  - converts 128-edge chunks into one-hot [128 x 128 dst] bf16 matrices on
    the vector engine (tensor_tensor is_equal against an iota constant).
    Since 16-aligned groups straddle 128-slot chunk boundaries, a chunk
    may span several dst windows: it gets one matmul task per window, with
    a separate host-built dstloc column masking foreign slots to -1,
  - matmul-accumulates transposed aggregates aggT[feat, dst] into a
    [128, 512] fp32 PSUM bank per direction with bf16 matmuls,
  - projects with pre-transposed, ALPHA-folded bf16 weights and adds the
    combined fp32 bias via the scalar engine,
  - stores the output transposed [128 feat, nodes] fp32; the host
    reassembles and un-permutes.
"""

from dataclasses import dataclass, field

import ml_dtypes
import numpy as np

import concourse.mybir as mybir
import concourse.tile as tile
from concourse import bacc
from concourse.bass_utils import run_bass_kernel_spmd

BF16 = ml_dtypes.bfloat16

P = 128
NCORE = 8
MACRO = 512
ALPHA = 0.5

NQUEUES = 4
SCRATCH = 49152
XG_BUFS = 2
OH_BUFS = 8

# micro.py bottleneck decomposition knobs
SKIP_GATHER = False
SKIP_COMPUTE = False
SINGLE_PACKET = False
USE_FP32 = False
USE_DUP = False  # duplicate bf16 rows to 512 B payloads
BALANCE = True   # degree-sorted round-robin node->core assignment
PADQ = 16        # per-(d,m,w,r) group padding quantum (16 or 128)


def roundup(a, b):
    return (a + b - 1) // b * b


@dataclass
class Meta:
    n_nodes: int
    span: int
    nmacro: int
    nw: int
    rspan: int
    nrange: int
    xrows: int
    totc16: int = 0
    totch: int = 0
    cmax: int = 0
    reps: int = 1
    sched: list = field(default_factory=list)
    node_pos: np.ndarray | None = None


def prep(x, edge_src, edge_dst):
    n = x.shape[0]
    span = roundup((n + NCORE - 1) // NCORE, P)
    nmacro = (span + MACRO - 1) // MACRO
    nw = MACRO // P
    xrows = roundup(n, 2048)
    nrange = 4
    rspan = xrows // nrange
    assert rspan <= 32768 and nrange * rspan == xrows
    meta = Meta(n_nodes=n, span=span, nmacro=nmacro, nw=nw, rspan=rspan,
                nrange=nrange, xrows=xrows)

    if USE_FP32:
        x_pad = np.zeros((xrows, P), dtype=np.float32)
        x_pad[:n] = x
    elif USE_DUP:
        x_pad = np.zeros((xrows, 2 * P), dtype=BF16)
        x_pad[:n, :P] = x.astype(BF16)
        x_pad[:n, P:] = x_pad[:n, :P]
    else:
        x_pad = np.zeros((xrows, P), dtype=BF16)
        x_pad[:n] = x.astype(BF16)

    es = np.asarray(edge_src, np.int64)
    ed = np.asarray(edge_dst, np.int64)

    # node -> global position (core = pos // span). Degree-sorted round-robin
    # dealing equalizes per-(d, m, w, r) counts across cores so the
    # max-over-cores padding stays tight.
    if BALANCE:
        deg = (np.bincount(es, minlength=n)
               + np.bincount(ed, minlength=n)).astype(np.int64)
        order = np.argsort(-deg, kind="stable")
        node_pos = np.empty(n, dtype=np.int64)
        ranks = np.arange(n, dtype=np.int64)
        node_pos[order] = (ranks % NCORE) * span + ranks // NCORE
    else:
        node_pos = np.arange(n, dtype=np.int64)
    meta.node_pos = node_pos

    key = [node_pos[ed], node_pos[es]]
    gat = [es, ed]

    counts = np.zeros((2, NCORE, nmacro, nw, nrange), dtype=np.int64)
    edge_groups = []
    for d in range(2):
        k, g = key[d], gat[d]
        core = k // span
        local = k - core * span
        m = local // MACRO
        w = (local % MACRO) // P
        r = g // rspan
        np.add.at(counts[d], (core, m, w, r), 1)
        comp = ((core * nmacro + m) * nw + w) * nrange + r
        order = np.argsort(comp, kind="stable")
        cs = comp[order]
        uniq, starts = np.unique(cs, return_index=True)
        ends = np.append(starts[1:], len(cs))
        edge_groups.append({int(u): order[s:e]
                            for u, s, e in zip(uniq, starts, ends)})

    gmax = counts.max(axis=1)
    S = np.where(gmax > 0, roundup(gmax, PADQ), 0)  # [2, nmacro, nw, nrange]

    NPDT = np.float32 if USE_FP32 else BF16

    # ---- layout pass: compute idx16/dstloc sizes and the schedule ----
    col16 = 0
    sched = []
    cmax = 0
    ntask_tot = 0
    for m in range(nmacro):
        byd = []
        for d in range(2):
            gathers = []  # (r, col16, crel0, nch, num_idxs)
            tasks = []    # (crel, w, r, off_w, c0slot)  c0slot call-relative
            crel = 0
            for r in range(nrange):
                sw = [int(S[d, m, w, r]) for w in range(nw)]
                num_idxs = sum(sw)
                if num_idxs == 0:
                    continue
                nch = -(-num_idxs // P)
                gathers.append((r, col16, crel, nch, num_idxs))
                offs = np.cumsum([0] + sw)
                for c_loc in range(nch):
                    s0, s1 = c_loc * P, min((c_loc + 1) * P, num_idxs)
                    for w in range(nw):
                        if sw[w] == 0:
                            continue
                        a = max(offs[w], s0)
                        b = min(offs[w + 1], s1)
                        if a < b:
                            tasks.append((crel + c_loc, w, r,
                                          int(offs[w]), s0))
                crel += nch
                col16 += num_idxs // 16
            byd.append({"gathers": gathers, "tasks": tasks})
            cmax = max(cmax, crel)
            ntask_tot += len(tasks)
        sched.append(byd)
    meta.totc16 = col16
    meta.totch = ntask_tot
    meta.cmax = cmax
    meta.sched = sched

    idx16 = np.zeros((NCORE, P, meta.totc16), dtype=np.int16)
    dstloc = np.full((NCORE, P, meta.totch), -1.0, dtype=NPDT)

    # ---- fill pass ----
    # per-(d,m,w,r,core): sorted gather indices and dst%128 values
    slot = 0
    for m in range(nmacro):
        for d in range(2):
            k, g = key[d], gat[d]
            sch = sched[m][d]
            # per (r, w, core): index stream and dst values
            cache = {}
            for r, c16_0, crel0, nch, num_idxs in sch["gathers"]:
                sw = [int(S[d, m, w, r]) for w in range(nw)]
                offs = np.cumsum([0] + sw)
                for c in range(NCORE):
                    stream = np.zeros(num_idxs, dtype=np.int16)
                    for w in range(nw):
                        if sw[w] == 0:
                            continue
                        u = ((c * nmacro + m) * nw + w) * nrange + r
                        eids = edge_groups[d].get(u, np.empty(0, np.int64))
                        eids = eids[np.argsort(g[eids], kind="stable")]
                        cnt = len(eids)
                        stream[offs[w]:offs[w] + cnt] = (
                            g[eids] - r * rspan).astype(np.int16)
                        dvals = np.full(sw[w], -1.0, dtype=NPDT)
                        dvals[:cnt] = (k[eids] % P).astype(NPDT)
                        cache[(r, w, c)] = dvals
                    wrapped = stream.reshape(num_idxs // 16, 16).T
                    idx16[c, :, c16_0:c16_0 + num_idxs // 16] = np.tile(
                        wrapped, (8, 1))
            for ti, (crel, w, r, off_w, s0) in enumerate(sch["tasks"]):
                swv = int(S[d, m, w, r])
                a = max(off_w, s0)
                b = min(off_w + swv, s0 + P)
                for c in range(NCORE):
                    dvals = cache[(r, w, c)]
                    dstloc[c, a - s0:b - s0, slot + ti] = (
                        dvals[a - off_w:b - off_w])
            slot += len(sch["tasks"])
    assert slot == meta.totch
    return meta, x_pad, idx16, dstloc


def build(meta: Meta):
    nc = bacc.Bacc("TRN2", target_bir_lowering=False,
                   num_swdge_queues=NQUEUES,
                   dynamic_dma_scratch_size=SCRATCH)
    OUTW = meta.nmacro * MACRO
    MDT = mybir.dt.float32 if USE_FP32 else mybir.dt.bfloat16
    XW = P if (USE_FP32 or not USE_DUP) else 2 * P

    x_t = nc.dram_tensor("x_pad", [meta.xrows, XW], MDT,
                         kind="ExternalInput")
    idx_d = nc.dram_tensor("idx16", [P, meta.totc16], mybir.dt.int16,
                           kind="ExternalInput")
    dl_d = nc.dram_tensor("dstloc", [P, meta.totch], MDT,
                          kind="ExternalInput")
    iota_d = nc.dram_tensor("iota_c", [P, 4 * P], MDT,
                            kind="ExternalInput")
    w1_d = nc.dram_tensor("w1t", [P, P], MDT, kind="ExternalInput")
    w2_d = nc.dram_tensor("w2t", [P, P], MDT, kind="ExternalInput")
    b_d = nc.dram_tensor("biasc", [P, 1], mybir.dt.float32, kind="ExternalInput")
    out_d = nc.dram_tensor("out_t", [P, OUTW], mybir.dt.float32,
                           kind="ExternalOutput")

    with tile.TileContext(nc) as tc:
        with (
            tc.tile_pool(name="consts", bufs=1) as consts,
            tc.tile_pool(name="xg0", bufs=XG_BUFS) as xgp0,
            tc.tile_pool(name="xg1", bufs=XG_BUFS) as xgp1,
            tc.tile_pool(name="oh", bufs=OH_BUFS) as ohp,
            tc.tile_pool(name="sb", bufs=2) as sbp,
            tc.tile_pool(name="ps", bufs=2, space="PSUM") as psp,
        ):
            iota_t = consts.tile([P, 4, P], MDT)
            nc.sync.dma_start(iota_t[:].rearrange("p c f -> p (c f)"),
                              iota_d[:])
            w1_t = consts.tile([P, P], MDT)
            nc.sync.dma_start(w1_t[:], w1_d[:])
            w2_t = consts.tile([P, P], MDT)
            nc.sync.dma_start(w2_t[:], w2_d[:])
            b_t = consts.tile([P, 1], mybir.dt.float32)
            nc.sync.dma_start(b_t[:], b_d[:])
            zero_t = consts.tile([P, 512], MDT)
            nc.vector.memset(zero_t[:], 0.0)
            dl_t = consts.tile([P, meta.totch], MDT)
            nc.sync.dma_start(dl_t[:], dl_d[:])

            idxcols_m = []
            for m in range(meta.nmacro):
                c0 = min(g[1] for d in range(2)
                         for g in meta.sched[m][d]["gathers"])
                c1 = max(g[1] + g[4] // 16 for d in range(2)
                         for g in meta.sched[m][d]["gathers"])
                idxcols_m.append((c0, c1))
            idxw = max(c1 - c0 for c0, c1 in idxcols_m)

            for _rep in range(meta.reps):
                slot = 0
                for m in range(meta.nmacro):
                    mc0, mc1 = idxcols_m[m]
                    idx_t = sbp.tile([P, idxw], mybir.dt.int16, tag="idx")
                    nc.sync.dma_start(idx_t[:, :mc1 - mc0],
                                      idx_d[:, mc0:mc1])
                    xgs = []
                    for d in range(2):
                        sch = meta.sched[m][d]
                        xgp = xgp0 if d == 0 else xgp1
                        xg = xgp.tile([P, meta.cmax, XW], MDT,
                                      tag=f"xg{d}")
                        xgs.append(xg)
                        if _rep == 0 and m < XG_BUFS:
                            # first use of this pool buffer: clear so that
                            # never-gathered tail slots hold finite values
                            # (stale x rows thereafter); 0 * one-hot(-1) = 0.
                            nc.vector.memset(
                                xg[:].rearrange("p c f -> p (c f)"), 0.0)
                        if SKIP_GATHER:
                            nc.vector.memset(xg[:, 0, 0:1], 0.0)
                        for gi, (r, col16, crel0, nch, num_idxs) in enumerate(
                                sch["gathers"]):
                            if SKIP_GATHER:
                                continue
                            nc.gpsimd.dma_gather(
                                xg[:, crel0:crel0 + nch, :],
                                x_t[r * meta.rspan:(r + 1) * meta.rspan, :],
                                idx_t[:, col16 - mc0:
                                      col16 - mc0 + num_idxs // 16],
                                num_idxs, num_idxs, XW,
                                single_packet=SINGLE_PACKET,
                                queue_num=gi % NQUEUES)
                    if SKIP_COMPUTE:
                        continue
                    agg_sb = []
                    for d in range(2):
                        sch = meta.sched[m][d]
                        xg = xgs[d]
                        agg_ps = psp.tile([P, 512], mybir.dt.float32,
                                          space="PSUM", tag=f"agg{d}")
                        nc.tensor.matmul(agg_ps[:], zero_t[:, :P], zero_t[:],
                                         start=True, stop=False)
                        tasks = sch["tasks"]
                        C = len(tasks)
                        for b0 in range(0, C, 4):
                            bn = min(4, C - b0)
                            oh4 = ohp.tile([P, 4, P], MDT,
                                           tag="oh")
                            nc.vector.tensor_tensor(
                                out=oh4[:, :bn, :], in0=iota_t[:, :bn, :],
                                in1=dl_t[:, slot:slot + bn].to_broadcast(
                                    [P, bn, P]),
                                op=mybir.AluOpType.is_equal)
                            for kk in range(bn):
                                ci = b0 + kk
                                crel, w = tasks[ci][0], tasks[ci][1]
                                nc.tensor.matmul(
                                    agg_ps[:, w * P:(w + 1) * P],
                                    xg[:, crel, :P], oh4[:, kk, :],
                                    start=False, stop=(ci == C - 1))
                            slot += bn
                        a_sb = sbp.tile([P, 512], MDT,
                                        tag=f"agg_sb{d}")
                        nc.scalar.activation(
                            out=a_sb[:], in_=agg_ps[:],
                            func=mybir.ActivationFunctionType.Copy)
                        agg_sb.append(a_sb)

                    out_ps = psp.tile([P, 512], mybir.dt.float32,
                                      space="PSUM", tag="out_ps")
                    nc.tensor.matmul(out_ps[:], w1_t[:], agg_sb[0][:],
                                     start=True, stop=False)
                    nc.tensor.matmul(out_ps[:], w2_t[:], agg_sb[1][:],
                                     start=False, stop=True)
                    o_sb = sbp.tile([P, 512], mybir.dt.float32, tag="o_sb")
                    nc.scalar.activation(
                        out=o_sb[:], in_=out_ps[:],
                        func=mybir.ActivationFunctionType.Identity,
                        bias=b_t[:, :1])
                    nc.sync.dma_start(out_d[:, m * MACRO:(m + 1) * MACRO],
                                      o_sb[:])

    nc.compile()
    return nc


def make_inputs(meta, x_pad, idx16, dstloc, W_sd, b_sd, W_ds, b_ds):
    NPDT = np.float32 if USE_FP32 else BF16
    w1t = np.ascontiguousarray((ALPHA * np.asarray(W_sd)).T.astype(NPDT))
    w2t = np.ascontiguousarray(
        ((1.0 - ALPHA) * np.asarray(W_ds)).T.astype(NPDT))
    biasc = (ALPHA * np.asarray(b_sd)
             + (1.0 - ALPHA) * np.asarray(b_ds)).astype(np.float32)[:, None]
    iota_c = np.tile(np.arange(P, dtype=NPDT), (P, 4))
    return [{
        "x_pad": x_pad, "idx16": np.ascontiguousarray(idx16[c]),
        "dstloc": np.ascontiguousarray(dstloc[c]),
        "iota_c": iota_c, "w1t": w1t, "w2t": w2t,
        "biasc": np.ascontiguousarray(biasc),
    } for c in range(NCORE)]


def assemble(meta, results):
    parts = [results[c]["out_t"][:, :meta.span].T for c in range(NCORE)]
    full = np.concatenate(parts, axis=0)
    return np.ascontiguousarray(full[meta.node_pos])


def kernel(x, edge_src, edge_dst, W_sd, b_sd, W_ds, b_ds):
    meta, x_pad, idx16, dstloc = prep(
        np.asarray(x, dtype=np.float32), edge_src, edge_dst)
    nc = build(meta)
    in_maps = make_inputs(meta, x_pad, idx16, dstloc, W_sd, b_sd, W_ds, b_ds)
    res = run_bass_kernel_spmd(nc, in_maps, core_ids=list(range(NCORE)))
    return assemble(meta, res.results)


# revision 21
# speedup vs baseline: 1.2129x; 1.0642x over previous
"""DirectedEncoder GNN kernel for 8 Trainium2 NeuronCores.

out = ALPHA*(segment_sum(x[edge_src] by edge_dst) @ W_sd.T + b_sd)
    + (1-ALPHA)*(segment_sum(x[edge_dst] by edge_src) @ W_ds.T + b_ds)

Sharding: edges are grouped by destination node (direction 2 by source),
and destination nodes are assigned to the 8 cores by degree-sorted
round-robin dealing (node with global degree rank i goes to core i%8, slot
i//8), so each core owns a disjoint, degree-balanced set of output rows
and no cross-core reduction is needed. The host un-permutes the output.

The kernel is bounded by SWDGE gather descriptor generation on the Q7
(Pool) engine: ~1.6 ns/descriptor aggregate plus ~0.76 us per dma_gather
call, with DMA transfers hidden behind generation when all 4 SWDGE queues
are used (2 queues doubles runtime; 512 B duplicated rows stall queues).
Descriptor count therefore is the metric to minimize: rows are plain bf16
256 B payloads, and every (dir, macro, window-of-128, range) edge group is
padded to a multiple of only 16 indices (the idx16 wrap quantum), using
the max count across cores — degree balancing keeps that max tight. One
compiled SPMD program serves all 8 cores.

Per core the kernel:
  - gathers x rows per edge with dma_gather (int16 indices relative to one
    of four 25088-row source ranges, one call per (dir, macro, range), 4
    SWDGE queues round-robin),
  - converts 128-edge chunks into one-hot [128 x 128 dst] bf16 matrices on
    the vector engine (tensor_tensor is_equal against an iota constant).
    Since 16-aligned groups straddle 128-slot chunk boundaries, a chunk
    may span several dst windows: it gets one matmul task per window, with
    a separate host-built dstloc column masking foreign slots to -1,
  - matmul-accumulates transposed aggregates aggT[feat, dst] into a
    [128, 512] fp32 PSUM bank per direction with bf16 matmuls,
  - projects with pre-transposed, ALPHA-folded bf16 weights and adds the
    combined fp32 bias via the scalar engine,
  - stores the output transposed [128 feat, nodes] fp32; the host
    reassembles and un-permutes.
"""

from dataclasses import dataclass, field

import ml_dtypes
import numpy as np

import concourse.mybir as mybir
import concourse.tile as tile
from concourse import bacc
from concourse.bass_utils import run_bass_kernel_spmd

BF16 = ml_dtypes.bfloat16

P = 128
NCORE = 8
MACRO = 512
ALPHA = 0.5

NQUEUES = 4
SCRATCH = 98304
XG_BUFS = 2
OH_BUFS = 8

# micro.py bottleneck decomposition knobs
SKIP_GATHER = False
SKIP_COMPUTE = False
SINGLE_PACKET = False
USE_FP32 = False
USE_DUP = False  # duplicate bf16 rows to 512 B payloads
BALANCE = True   # degree-sorted round-robin node->core assignment
PADQ = 16        # per-(d,m,w,r) group padding quantum (16 or 128)


def roundup(a, b):
    return (a + b - 1) // b * b


@dataclass
class Meta:
    n_nodes: int
    span: int
    nmacro: int
    nw: int
    rspan: int
    nrange: int
    xrows: int
    totc16: int = 0
    totch: int = 0
    cmax: int = 0
    reps: int = 1
    sched: list = field(default_factory=list)
    node_pos: np.ndarray | None = None


def prep(x, edge_src, edge_dst):
    n = x.shape[0]
    span = roundup((n + NCORE - 1) // NCORE, P)
    nmacro = (span + MACRO - 1) // MACRO
    nw = MACRO // P
    xrows = roundup(n, 2048)
    nrange = 4
    rspan = xrows // nrange
    assert rspan <= 32768 and nrange * rspan == xrows
    meta = Meta(n_nodes=n, span=span, nmacro=nmacro, nw=nw, rspan=rspan,
                nrange=nrange, xrows=xrows)

    if USE_FP32:
        x_pad = np.zeros((xrows, P), dtype=np.float32)
        x_pad[:n] = x
    elif USE_DUP:
        x_pad = np.zeros((xrows, 2 * P), dtype=BF16)
        x_pad[:n, :P] = x.astype(BF16)
        x_pad[:n, P:] = x_pad[:n, :P]
    else:
        x_pad = np.zeros((xrows, P), dtype=BF16)
        x_pad[:n] = x.astype(BF16)

    es = np.asarray(edge_src, np.int64)
    ed = np.asarray(edge_dst, np.int64)

    # node -> global position (core = pos // span). Degree-sorted round-robin
    # dealing equalizes per-(d, m, w, r) counts across cores so the
    # max-over-cores padding stays tight.
    if BALANCE:
        deg = (np.bincount(es, minlength=n)
               + np.bincount(ed, minlength=n)).astype(np.int64)
        order = np.argsort(-deg, kind="stable")
        node_pos = np.empty(n, dtype=np.int64)
        ranks = np.arange(n, dtype=np.int64)
        node_pos[order] = (ranks % NCORE) * span + ranks // NCORE
    else:
        node_pos = np.arange(n, dtype=np.int64)
    meta.node_pos = node_pos

    key = [node_pos[ed], node_pos[es]]
    gat = [es, ed]

    counts = np.zeros((2, NCORE, nmacro, nw, nrange), dtype=np.int64)
    edge_groups = []
    for d in range(2):
        k, g = key[d], gat[d]
        core = k // span
        local = k - core * span
        m = local // MACRO
        w = (local % MACRO) // P
        r = g // rspan
        np.add.at(counts[d], (core, m, w, r), 1)
        comp = ((core * nmacro + m) * nw + w) * nrange + r
        order = np.argsort(comp, kind="stable")
        cs = comp[order]
        uniq, starts = np.unique(cs, return_index=True)
        ends = np.append(starts[1:], len(cs))
        edge_groups.append({int(u): order[s:e]
                            for u, s, e in zip(uniq, starts, ends)})

    gmax = counts.max(axis=1)
    S = np.where(gmax > 0, roundup(gmax, PADQ), 0)  # [2, nmacro, nw, nrange]

    NPDT = np.float32 if USE_FP32 else BF16

    # ---- layout pass: compute idx16/dstloc sizes and the schedule ----
    col16 = 0
    sched = []
    cmax = 0
    ntask_tot = 0
    for m in range(nmacro):
        byd = []
        for d in range(2):
            gathers = []  # (r, col16, crel0, nch, num_idxs)
            tasks = []    # (crel, w, r, off_w, c0slot)  c0slot call-relative
            crel = 0
            for r in range(nrange):
                sw = [int(S[d, m, w, r]) for w in range(nw)]
                num_idxs = sum(sw)
                if num_idxs == 0:
                    continue
                nch = -(-num_idxs // P)
                gathers.append((r, col16, crel, nch, num_idxs))
                offs = np.cumsum([0] + sw)
                for c_loc in range(nch):
                    s0, s1 = c_loc * P, min((c_loc + 1) * P, num_idxs)
                    for w in range(nw):
                        if sw[w] == 0:
                            continue
                        a = max(offs[w], s0)
                        b = min(offs[w + 1], s1)
                        if a < b:
                            tasks.append((crel + c_loc, w, r,
                                          int(offs[w]), s0))
                crel += nch
                col16 += num_idxs // 16
            byd.append({"gathers": gathers, "tasks": tasks})
            cmax = max(cmax, crel)
            ntask_tot += len(tasks)
        sched.append(byd)
    meta.totc16 = col16
    meta.totch = ntask_tot
    meta.cmax = cmax
    meta.sched = sched

    idx16 = np.zeros((NCORE, P, meta.totc16), dtype=np.int16)
    dstloc = np.full((NCORE, P, meta.totch), -1.0, dtype=NPDT)

    # ---- fill pass ----
    # per-(d,m,w,r,core): sorted gather indices and dst%128 values
    slot = 0
    for m in range(nmacro):
        for d in range(2):
            k, g = key[d], gat[d]
            sch = sched[m][d]
            # per (r, w, core): index stream and dst values
            cache = {}
            for r, c16_0, crel0, nch, num_idxs in sch["gathers"]:
                sw = [int(S[d, m, w, r]) for w in range(nw)]
                offs = np.cumsum([0] + sw)
                for c in range(NCORE):
                    stream = np.zeros(num_idxs, dtype=np.int16)
                    for w in range(nw):
                        if sw[w] == 0:
                            continue
                        u = ((c * nmacro + m) * nw + w) * nrange + r
                        eids = edge_groups[d].get(u, np.empty(0, np.int64))
                        eids = eids[np.argsort(g[eids], kind="stable")]
                        cnt = len(eids)
                        stream[offs[w]:offs[w] + cnt] = (
                            g[eids] - r * rspan).astype(np.int16)
                        dvals = np.full(sw[w], -1.0, dtype=NPDT)
                        dvals[:cnt] = (k[eids] % P).astype(NPDT)
                        cache[(r, w, c)] = dvals
                    wrapped = stream.reshape(num_idxs // 16, 16).T
                    idx16[c, :, c16_0:c16_0 + num_idxs // 16] = np.tile(
                        wrapped, (8, 1))
            for ti, (crel, w, r, off_w, s0) in enumerate(sch["tasks"]):
                swv = int(S[d, m, w, r])
                a = max(off_w, s0)
                b = min(off_w + swv, s0 + P)
                for c in range(NCORE):
                    dvals = cache[(r, w, c)]
                    dstloc[c, a - s0:b - s0, slot + ti] = (
                        dvals[a - off_w:b - off_w])
            slot += len(sch["tasks"])
    assert slot == meta.totch
    return meta, x_pad, idx16, dstloc


def build(meta: Meta):
    nc = bacc.Bacc("TRN2", target_bir_lowering=False,
                   num_swdge_queues=NQUEUES,
                   dynamic_dma_scratch_size=SCRATCH)
    OUTW = meta.nmacro * MACRO
    MDT = mybir.dt.float32 if USE_FP32 else mybir.dt.bfloat16
    XW = P if (USE_FP32 or not USE_DUP) else 2 * P

    x_t = nc.dram_tensor("x_pad", [meta.xrows, XW], MDT,
                         kind="ExternalInput")
    idx_d = nc.dram_tensor("idx16", [P, meta.totc16], mybir.dt.int16,
                           kind="ExternalInput")
    dl_d = nc.dram_tensor("dstloc", [P, meta.totch], MDT,
                          kind="ExternalInput")
    iota_d = nc.dram_tensor("iota_c", [P, 8 * P], MDT,
                            kind="ExternalInput")
    w1_d = nc.dram_tensor("w1t", [P, P], MDT, kind="ExternalInput")
    w2_d = nc.dram_tensor("w2t", [P, P], MDT, kind="ExternalInput")
    b_d = nc.dram_tensor("biasc", [P, 1], mybir.dt.float32, kind="ExternalInput")
    out_d = nc.dram_tensor("out_t", [P, OUTW], mybir.dt.float32,
                           kind="ExternalOutput")

    with tile.TileContext(nc) as tc:
        with (
            tc.tile_pool(name="consts", bufs=1) as consts,
            tc.tile_pool(name="xg0", bufs=XG_BUFS) as xgp0,
            tc.tile_pool(name="xg1", bufs=XG_BUFS) as xgp1,
            tc.tile_pool(name="oh", bufs=OH_BUFS) as ohp,
            tc.tile_pool(name="sb", bufs=2) as sbp,
            tc.tile_pool(name="ps", bufs=2, space="PSUM") as psp,
        ):
            iota_t = consts.tile([P, 8, P], MDT)
            nc.sync.dma_start(iota_t[:].rearrange("p c f -> p (c f)"),
                              iota_d[:])
            w1_t = consts.tile([P, P], MDT)
            nc.sync.dma_start(w1_t[:], w1_d[:])
            w2_t = consts.tile([P, P], MDT)
            nc.sync.dma_start(w2_t[:], w2_d[:])
            b_t = consts.tile([P, 1], mybir.dt.float32)
            nc.sync.dma_start(b_t[:], b_d[:])
            zero_t = consts.tile([P, 512], MDT)
            nc.vector.memset(zero_t[:], 0.0)
            dl_t = consts.tile([P, meta.totch], MDT)
            nc.sync.dma_start(dl_t[:], dl_d[:])

            idxcols_m = []
            for m in range(meta.nmacro):
                c0 = min(g[1] for d in range(2)
                         for g in meta.sched[m][d]["gathers"])
                c1 = max(g[1] + g[4] // 16 for d in range(2)
                         for g in meta.sched[m][d]["gathers"])
                idxcols_m.append((c0, c1))
            idxw = max(c1 - c0 for c0, c1 in idxcols_m)

            for _rep in range(meta.reps):
                slot = 0
                for m in range(meta.nmacro):
                    mc0, mc1 = idxcols_m[m]
                    idx_t = sbp.tile([P, idxw], mybir.dt.int16, tag="idx")
                    nc.sync.dma_start(idx_t[:, :mc1 - mc0],
                                      idx_d[:, mc0:mc1])
                    xgs = []
                    for d in range(2):
                        sch = meta.sched[m][d]
                        xgp = xgp0 if d == 0 else xgp1
                        xg = xgp.tile([P, meta.cmax, XW], MDT,
                                      tag=f"xg{d}")
                        xgs.append(xg)
                        if _rep == 0 and m < XG_BUFS:
                            # first use of this pool buffer: clear so that
                            # never-gathered tail slots hold finite values
                            # (stale x rows thereafter); 0 * one-hot(-1) = 0.
                            nc.vector.memset(
                                xg[:].rearrange("p c f -> p (c f)"), 0.0)
                        if SKIP_GATHER:
                            nc.vector.memset(xg[:, 0, 0:1], 0.0)
                        for gi, (r, col16, crel0, nch, num_idxs) in enumerate(
                                sch["gathers"]):
                            if SKIP_GATHER:
                                continue
                            nc.gpsimd.dma_gather(
                                xg[:, crel0:crel0 + nch, :],
                                x_t[r * meta.rspan:(r + 1) * meta.rspan, :],
                                idx_t[:, col16 - mc0:
                                      col16 - mc0 + num_idxs // 16],
                                num_idxs, num_idxs, XW,
                                single_packet=SINGLE_PACKET,
                                queue_num=gi % NQUEUES)
                    if SKIP_COMPUTE:
                        continue
                    agg_sb = []
                    for d in range(2):
                        sch = meta.sched[m][d]
                        xg = xgs[d]
                        agg_ps = psp.tile([P, 512], mybir.dt.float32,
                                          space="PSUM", tag=f"agg{d}")
                        nc.tensor.matmul(agg_ps[:], zero_t[:, :P], zero_t[:],
                                         start=True, stop=False)
                        tasks = sch["tasks"]
                        C = len(tasks)
                        for b0 in range(0, C, 8):
                            bn = min(8, C - b0)
                            oh4 = ohp.tile([P, 8, P], MDT,
                                           tag="oh")
                            nc.vector.tensor_tensor(
                                out=oh4[:, :bn, :], in0=iota_t[:, :bn, :],
                                in1=dl_t[:, slot:slot + bn].to_broadcast(
                                    [P, bn, P]),
                                op=mybir.AluOpType.is_equal)
                            for kk in range(bn):
                                ci = b0 + kk
                                crel, w = tasks[ci][0], tasks[ci][1]
                                nc.tensor.matmul(
                                    agg_ps[:, w * P:(w + 1) * P],
                                    xg[:, crel, :P], oh4[:, kk, :],
                                    start=False, stop=(ci == C - 1))
                            slot += bn
                        a_sb = sbp.tile([P, 512], MDT,
                                        tag=f"agg_sb{d}")
                        nc.scalar.activation(
                            out=a_sb[:], in_=agg_ps[:],
                            func=mybir.ActivationFunctionType.Copy)
                        agg_sb.append(a_sb)

                    out_ps = psp.tile([P, 512], mybir.dt.float32,
                                      space="PSUM", tag="out_ps")
                    nc.tensor.matmul(out_ps[:], w1_t[:], agg_sb[0][:],
                                     start=True, stop=False)
                    nc.tensor.matmul(out_ps[:], w2_t[:], agg_sb[1][:],
                                     start=False, stop=True)
                    o_sb = sbp.tile([P, 512], mybir.dt.float32, tag="o_sb")
                    nc.scalar.activation(
                        out=o_sb[:], in_=out_ps[:],
                        func=mybir.ActivationFunctionType.Identity,
                        bias=b_t[:, :1])
                    nc.sync.dma_start(out_d[:, m * MACRO:(m + 1) * MACRO],
                                      o_sb[:])

    nc.compile()
    return nc


def make_inputs(meta, x_pad, idx16, dstloc, W_sd, b_sd, W_ds, b_ds):
    NPDT = np.float32 if USE_FP32 else BF16
    w1t = np.ascontiguousarray((ALPHA * np.asarray(W_sd)).T.astype(NPDT))
    w2t = np.ascontiguousarray(
        ((1.0 - ALPHA) * np.asarray(W_ds)).T.astype(NPDT))
    biasc = (ALPHA * np.asarray(b_sd)
             + (1.0 - ALPHA) * np.asarray(b_ds)).astype(np.float32)[:, None]
    iota_c = np.tile(np.arange(P, dtype=NPDT), (P, 8))
    return [{
        "x_pad": x_pad, "idx16": np.ascontiguousarray(idx16[c]),
        "dstloc": np.ascontiguousarray(dstloc[c]),
        "iota_c": iota_c, "w1t": w1t, "w2t": w2t,
        "biasc": np.ascontiguousarray(biasc),
    } for c in range(NCORE)]


def assemble(meta, results):
    parts = [results[c]["out_t"][:, :meta.span].T for c in range(NCORE)]
    full = np.concatenate(parts, axis=0)
    return np.ascontiguousarray(full[meta.node_pos])


def kernel(x, edge_src, edge_dst, W_sd, b_sd, W_ds, b_ds):
    meta, x_pad, idx16, dstloc = prep(
        np.asarray(x, dtype=np.float32), edge_src, edge_dst)
    nc = build(meta)
    in_maps = make_inputs(meta, x_pad, idx16, dstloc, W_sd, b_sd, W_ds, b_ds)
    res = run_bass_kernel_spmd(nc, in_maps, core_ids=list(range(NCORE)))
    return assemble(meta, res.results)
